# revision 8
# baseline (speedup 1.0000x reference)
"""Radius-graph adjacency mask (radius_graph r=3, loop=True) on 8 TRN2 NeuronCores.

Strategy
--------
mask[i, j] = (||p_i - p_j||^2 <= R2)  for pos [8192, 3].

val(i, j) = (R2 + eps) - d2(i, j) is computed as a single small-K matmul:
    val = sum_r q_rows[r, i] * k_rows[r, j]
where the q/k rows hold 3-way bf16 splits of the augmented query/key vectors
(2x, sq terms), so the bf16 TensorE matmul (1 cycle/row) reproduces the fp32
value to ~24-bit accuracy.  PSUM then holds val; mask = (val >= 0) via
VectorE is_ge / ScalarE Sign (both engines share the PSUM-read load), written
as int8 and DMA'd out.

Sharding: rows data-parallel across 8 cores (1024 query rows each).  In sorted
mode the atoms are z-sorted so each 128-query block only needs a W-wide window
of keys (all |z_i - z_j| <= 3 neighbors are inside), cutting the per-core slab
from [1024, 8192] to [1024, W].  The host scatters the slabs back into the
full [8192, 8192] bool mask.
"""

from contextlib import ExitStack

import ml_dtypes
import numpy as np

import concourse.bass as bass
import concourse.mybir as mybir
import concourse.tile as tile
from concourse import bacc
from concourse.bass_utils import run_bass_kernel_spmd

N = 8192
R2 = 9.0
RADIUS = 3.0
EPS = 1e-5
NCORES = 8
P = 128
KP = 32                       # padded contraction rows (30 used)
BLOCKS = (N // NCORES) // P   # 8 query blocks of 128 rows per core
BF16 = ml_dtypes.bfloat16

SORT_MODE = True              # z-sorted banded windows (falls back to dense)


def _bf16_split3(x):
    """Split f64 array into 3 bf16 components summing to ~24-bit accuracy."""
    b0 = x.astype(BF16)
    r1 = x - b0.astype(np.float64)
    b1 = r1.astype(BF16)
    r2 = r1 - b1.astype(np.float64)
    b2 = r2.astype(BF16)
    return b0.astype(np.float64), b1.astype(np.float64), b2.astype(np.float64)


def _build_rows(ps):
    """Build the KP-row augmented query/key matrices (f64 holding bf16 values).

    val = sum_r q_rows[r, i] * k_rows[r, j] = (R2 + EPS) - d2(i, j)
    """
    n = ps.shape[0]
    A = 2.0 * ps.T                      # (3, n) query-side coefficient
    B = ps.T                            # (3, n) key-side
    S = (R2 + EPS) - (ps * ps).sum(1)   # query-side constant term
    T = -(ps * ps).sum(1)               # key-side constant term
    ones = np.ones(n)

    rows_q, rows_k = [], []
    for c in range(3):
        Asp = _bf16_split3(A[c])
        Bsp = _bf16_split3(B[c])
        # all split-product terms above ~2^-32 relative (drop (2,2) only)
        for u, v in [(0, 0), (0, 1), (1, 0), (1, 1), (0, 2), (2, 0), (1, 2), (2, 1)]:
            rows_q.append(Asp[u])
            rows_k.append(Bsp[v])
    for s in _bf16_split3(S):
        rows_q.append(s)
        rows_k.append(ones)
    for t in _bf16_split3(T):
        rows_q.append(ones)
        rows_k.append(t)

    q = np.zeros((KP, n))
    k = np.zeros((KP, n))
    q[: len(rows_q)] = np.stack(rows_q)
    k[: len(rows_k)] = np.stack(rows_k)
    return q, k


def _build_graph(W, nslab, slab_of_block):
    """Build the SPMD Bass graph (same for every core).

    Inputs (per core):
      q   [128, BLOCKS, 128] bf16 : partition 32*j + r holds q_rows[r] for the
                                    block's queries, replicated over j=0..3.
      k   [128, nslab, W//4] bf16 : partition 32*j + r holds k_rows[r] for
                                    column quarter j of the slab window.
    Output:
      out [BLOCKS, 128, W] int8   : 1 where mask else 0/-1 (decode == 1).
    """
    WQ = W // 4          # columns per quarter
    PSUM_FD = 2048       # psum tile free dim (4 banks)
    assert W % 2048 == 0 and WQ % 512 == 0

    nc = bacc.Bacc("TRN2", target_bir_lowering=False)
    q_ext = nc.declare_dram_parameter("q", [P, BLOCKS, P], mybir.dt.bfloat16, isOutput=False)
    k_ext = nc.declare_dram_parameter("k", [P, nslab, WQ], mybir.dt.bfloat16, isOutput=False)
    out_ext = nc.declare_dram_parameter("out", [BLOCKS, P, W], mybir.dt.int8, isOutput=True)

    with tile.TileContext(nc) as tc, ExitStack() as ctx:
        kpool = ctx.enter_context(tc.tile_pool(name="keys", bufs=1))
        qpool = ctx.enter_context(tc.tile_pool(name="queries", bufs=1))
        psum = ctx.enter_context(tc.tile_pool(name="psum", bufs=2, space="PSUM"))
        mpool = ctx.enter_context(tc.tile_pool(name="mask", bufs=4))

        q_sb = qpool.tile([P, BLOCKS, P], mybir.dt.bfloat16)
        nc.gpsimd.dma_start(q_sb[:], q_ext[:])
        k_sb = kpool.tile([P, nslab, WQ], mybir.dt.bfloat16)
        if nslab > 1:
            nc.gpsimd.dma_start(k_sb[:, :2], k_ext[:, :2])
            nc.gpsimd.dma_start(k_sb[:, 2:], k_ext[:, 2:])
        else:
            nc.gpsimd.dma_start(k_sb[:], k_ext[:])

        tiles_per_block = W // PSUM_FD
        tidx = 0
        mt = None
        for b in range(BLOCKS):
            s = slab_of_block[b]
            for h in range(tiles_per_block):
                pt = psum.tile([P, PSUM_FD], mybir.dt.float32)
                for t2 in range(PSUM_FD // 512):
                    col0 = h * PSUM_FD + t2 * 512   # column offset within W
                    j = col0 // WQ                  # quarter -> PE row group
                    qcol = col0 % WQ
                    nc.tensor.matmul(
                        pt[:, t2 * 512 : (t2 + 1) * 512],
                        lhsT=q_sb[32 * j : 32 * (j + 1), b, :],
                        rhs=k_sb[32 * j : 32 * (j + 1), s, qcol : qcol + 512],
                        start=True,
                        stop=True,
                        tile_position=(32 * j, 0),
                    )
                # pair two consecutive tiles into one SBUF tile / one output DMA
                if tidx % 2 == 0:
                    mt = mpool.tile([P, 2, PSUM_FD], mybir.dt.int8)
                    nc.scalar.activation(mt[:, 0], pt[:], mybir.ActivationFunctionType.Sign)
                else:
                    nc.vector.tensor_scalar(mt[:, 1], pt[:], 0.0, None, mybir.AluOpType.is_ge)
                    if tiles_per_block == 1:
                        nc.sync.dma_start(
                            out_ext[b - 1 : b + 1].rearrange("b p w -> p b w"), mt[:]
                        )
                    else:
                        nc.sync.dma_start(
                            out_ext[b, :, (h - 1) * PSUM_FD : (h + 1) * PSUM_FD],
                            mt[:].rearrange("p b w -> p (b w)"),
                        )
                tidx += 1
    nc.compile()
    return nc


def _quarters(k32):
    """[32, W] -> [128, W//4] with quarter j at partitions 32j..32j+31."""
    W = k32.shape[1]
    return k32.reshape(KP, 4, W // 4).transpose(1, 0, 2).reshape(P, W // 4)


def _prepare(pos):
    """Host prep: sort, windows, split rows, per-core in_maps."""
    posf = np.asarray(pos, dtype=np.float64)
    nblocks = N // P

    # recenter: d2 is translation-invariant, but smaller |coords| shrink the
    # fp32 cancellation error in sq_i + sq_j - 2 x.y by ~4x
    posf = posf - (posf.min(0) + posf.max(0)) / 2.0

    use_sort = SORT_MODE
    if use_sort:
        order = np.argsort(posf[:, 2], kind="stable")
        ps = posf[order]
        z = ps[:, 2]
        zb = z.reshape(nblocks, P)
        ilo = np.searchsorted(z, zb.min(1) - RADIUS, side="left")
        ihi = np.searchsorted(z, zb.max(1) + RADIUS, side="right")
        wmax = int((ihi - ilo).max())
        W = max(2048, -(-wmax // 2048) * 2048)
        if W >= N:
            use_sort = False
    if not use_sort:
        order = np.arange(N)
        ps = posf
        W = N
        ilo = np.zeros(nblocks, dtype=np.int64)

    off = np.clip(ilo, 0, N - W).astype(np.int64)
    qrows, krows = _build_rows(ps)           # (32, N) f64 over sorted order
    q16 = qrows.astype(BF16)
    k16 = krows.astype(BF16)

    nslab = BLOCKS if use_sort else 1
    in_maps = []
    for c in range(NCORES):
        qc = np.zeros((P, BLOCKS, P), dtype=BF16)
        kc = np.zeros((P, nslab, W // 4), dtype=BF16)
        for b in range(BLOCKS):
            g = c * BLOCKS + b
            qb = q16[:, g * P : (g + 1) * P]          # [32, 128]
            qc[:, b, :] = np.tile(qb, (4, 1))         # replicate to 4 groups
            if use_sort:
                kc[:, b, :] = _quarters(k16[:, off[g] : off[g] + W])
        if not use_sort:
            kc[:, 0, :] = _quarters(k16)
        in_maps.append({"q": qc, "k": kc})

    slab_of_block = list(range(BLOCKS)) if use_sort else [0] * BLOCKS
    return order, off, W, nslab, slab_of_block, in_maps


LAST_RESULTS = None  # BassKernelResults of the most recent run (for profiling)


def kernel(pos):
    global LAST_RESULTS
    order, off, W, nslab, slab_of_block, in_maps = _prepare(pos)
    nc = _build_graph(W, nslab, slab_of_block)
    res = run_bass_kernel_spmd(nc, in_maps, list(range(NCORES)))
    LAST_RESULTS = res

    full = np.zeros((N, N), dtype=bool)
    for c in range(NCORES):
        o = res.results[c]["out"]                      # [BLOCKS, 128, W] int8
        for b in range(BLOCKS):
            g = c * BLOCKS + b
            rows = order[g * P : (g + 1) * P]
            cols = order[off[g] : off[g] + W]
            full[np.ix_(rows, cols)] = o[b] == 1
    return full


# revision 10
# speedup vs baseline: 1.3527x; 1.3527x over previous
"""Radius-graph adjacency mask (radius_graph r=3, loop=True) on 8 TRN2 NeuronCores.

Strategy
--------
mask[i, j] = (||p_i - p_j||^2 <= R2)  for pos [8192, 3].

val(i, j) = (R2 + eps) - d2(i, j) is computed as a single small-K matmul:
    val = sum_r q_rows[r, i] * k_rows[r, j]
where the q/k rows hold 3-way bf16 splits of the augmented query/key vectors
(2x, sq terms), so the bf16 TensorE matmul (1 cycle/row) reproduces the fp32
value to ~24-bit accuracy.  PSUM then holds val; mask = (val >= 0) via
VectorE is_ge / ScalarE Sign (both engines share the PSUM-read load), written
as int8 and DMA'd out.

Sharding: rows data-parallel across 8 cores (1024 query rows each).  Atoms are
z-sorted; in symmetric mode each 128-query block only computes keys at sorted
index >= its own start inside the z-window (all |z_i - z_j| <= 3 forward
neighbors), a W~1024 slab, and the host mirrors the lower triangle.  The host
scatters the slabs back into the full [8192, 8192] bool mask.
"""

from contextlib import ExitStack

import ml_dtypes
import numpy as np

import concourse.bass as bass
import concourse.mybir as mybir
import concourse.tile as tile
from concourse import bacc
from concourse.bass_utils import run_bass_kernel_spmd

N = 8192
R2 = 9.0
RADIUS = 3.0
EPS = 1e-5
NCORES = 8
P = 128
KP = 32                       # padded contraction rows (30 used)
BLOCKS = (N // NCORES) // P   # 8 query blocks of 128 rows per core
BF16 = ml_dtypes.bfloat16


def _bf16_split3(x):
    """Split f64 array into 3 bf16 components summing to ~24-bit accuracy."""
    b0 = x.astype(BF16)
    r1 = x - b0.astype(np.float64)
    b1 = r1.astype(BF16)
    r2 = r1 - b1.astype(np.float64)
    b2 = r2.astype(BF16)
    return b0.astype(np.float64), b1.astype(np.float64), b2.astype(np.float64)


def _build_rows(ps):
    """Build the KP-row augmented query/key matrices (f64 holding bf16 values).

    val = sum_r q_rows[r, i] * k_rows[r, j] = (R2 + EPS) - d2(i, j)
    """
    n = ps.shape[0]
    A = 2.0 * ps.T                      # (3, n) query-side coefficient
    B = ps.T                            # (3, n) key-side
    S = (R2 + EPS) - (ps * ps).sum(1)   # query-side constant term
    T = -(ps * ps).sum(1)               # key-side constant term
    ones = np.ones(n)

    rows_q, rows_k = [], []
    for c in range(3):
        Asp = _bf16_split3(A[c])
        Bsp = _bf16_split3(B[c])
        # all split-product terms above ~2^-32 relative (drop (2,2) only)
        for u, v in [(0, 0), (0, 1), (1, 0), (1, 1), (0, 2), (2, 0), (1, 2), (2, 1)]:
            rows_q.append(Asp[u])
            rows_k.append(Bsp[v])
    for s in _bf16_split3(S):
        rows_q.append(s)
        rows_k.append(ones)
    for t in _bf16_split3(T):
        rows_q.append(ones)
        rows_k.append(t)

    q = np.zeros((KP, n))
    k = np.zeros((KP, n))
    q[: len(rows_q)] = np.stack(rows_q)
    k[: len(rows_k)] = np.stack(rows_k)
    return q, k


def _build_graph(W, nslab, slab_of_block):
    """Build the SPMD Bass graph (same for every core).

    Column chunk c (of W//512) lives at PE row group c%4 (partitions 32(c%4)..),
    free offset (c//4)*512 -- so DMA fills all 128 partitions and up to 4
    matmuls run concurrently on distinct row groups.

    Inputs (per core):
      q   [128, BLOCKS, 128] bf16 : partition 32*g + r holds q_rows[r] for the
                                    block's queries, replicated over g=0..3.
      k   [128, nslab, WQ] bf16   : partition 32*g + r holds k_rows[r] for
                                    column chunks c%4==g of the slab window.
    Output:
      out [BLOCKS, 128, W] int8   : 1 where mask else 0/-1 (decode == 1).
    """
    CH = W // 512                      # column chunks per slab
    WQ = 512 * ((CH + 3) // 4)         # per-group free size
    PSUM_FD = W if W <= 2048 else 2048
    assert W % 512 == 0 and W % PSUM_FD == 0
    tiles_per_block = W // PSUM_FD
    GP = 2 if W >= 2048 else 4         # blocks (or tiles) per output DMA group

    nc = bacc.Bacc("TRN2", target_bir_lowering=False)
    q_ext = nc.declare_dram_parameter("q", [P, BLOCKS, P], mybir.dt.bfloat16, isOutput=False)
    k_ext = nc.declare_dram_parameter("k", [P, nslab, WQ], mybir.dt.bfloat16, isOutput=False)
    out_ext = nc.declare_dram_parameter("out", [BLOCKS, P, W], mybir.dt.int8, isOutput=True)

    with tile.TileContext(nc) as tc, ExitStack() as ctx:
        kpool = ctx.enter_context(tc.tile_pool(name="keys", bufs=1))
        qpool = ctx.enter_context(tc.tile_pool(name="queries", bufs=1))
        psum = ctx.enter_context(
            tc.tile_pool(name="psum", bufs=8 // (PSUM_FD // 512), space="PSUM")
        )
        mpool = ctx.enter_context(tc.tile_pool(name="mask", bufs=2))

        # inputs on two different DMA-capable engines so neither blocks the other
        q_sb = qpool.tile([P, BLOCKS, P], mybir.dt.bfloat16)
        nc.sync.dma_start(q_sb[:], q_ext[:])
        k_sb = kpool.tile([P, nslab, WQ], mybir.dt.bfloat16)
        nc.scalar.dma_start(k_sb[:], k_ext[:])

        tidx = 0
        mt = None
        for b in range(BLOCKS):
            s = slab_of_block[b]
            for h in range(tiles_per_block):
                pt = psum.tile([P, PSUM_FD], mybir.dt.float32)
                for t2 in range(PSUM_FD // 512):
                    c = h * (PSUM_FD // 512) + t2   # column chunk within W
                    g = c % 4                       # PE row group
                    qcol = (c // 4) * 512
                    nc.tensor.matmul(
                        pt[:, t2 * 512 : (t2 + 1) * 512],
                        lhsT=q_sb[32 * g : 32 * (g + 1), b, :],
                        rhs=k_sb[32 * g : 32 * (g + 1), s, qcol : qcol + 512],
                        start=True,
                        stop=True,
                        tile_position=(32 * g, 0),
                    )
                # group GP consecutive tiles into one SBUF tile / one output DMA
                gslot = tidx % GP
                if gslot == 0:
                    mt = mpool.tile([P, GP, PSUM_FD], mybir.dt.int8)
                if tidx % 2 == 0:
                    nc.scalar.activation(mt[:, gslot], pt[:], mybir.ActivationFunctionType.Sign)
                else:
                    nc.vector.tensor_scalar(mt[:, gslot], pt[:], 0.0, None, mybir.AluOpType.is_ge)
                if gslot == GP - 1:
                    if tiles_per_block == 1:
                        b0 = b - GP + 1
                        nc.sync.dma_start(
                            out_ext[b0 : b + 1, :, :].rearrange("b p w -> p b w"),
                            mt[:],
                        )
                    else:
                        h0 = h - GP + 1
                        nc.sync.dma_start(
                            out_ext[b, :, h0 * PSUM_FD : (h + 1) * PSUM_FD],
                            mt[:].rearrange("p b w -> p (b w)"),
                        )
                tidx += 1
    nc.compile()
    return nc


def _chunk_layout(k32, WQ):
    """[32, W] -> [128, WQ]: chunk c (512 cols) at partitions 32*(c%4), offset (c//4)*512."""
    W = k32.shape[1]
    CH = W // 512
    out = np.zeros((P, WQ), dtype=k32.dtype)
    for c in range(CH):
        g, m = c % 4, c // 4
        out[32 * g : 32 * (g + 1), m * 512 : (m + 1) * 512] = k32[:, c * 512 : (c + 1) * 512]
    return out


def _prepare(pos):
    """Host prep: sort, windows, split rows, per-core in_maps."""
    posf = np.asarray(pos, dtype=np.float64)
    nblocks = N // P

    # recenter: d2 is translation-invariant, but smaller |coords| shrink the
    # fp32 cancellation error in sq_i + sq_j - 2 x.y by ~4x
    posf = posf - (posf.min(0) + posf.max(0)) / 2.0

    order = np.argsort(posf[:, 2], kind="stable")
    ps = posf[order]
    z = ps[:, 2]
    zb = z.reshape(nblocks, P)
    ilo = np.searchsorted(z, zb.min(1) - RADIUS, side="left")
    ihi = np.searchsorted(z, zb.max(1) + RADIUS, side="right")

    starts = np.arange(nblocks, dtype=np.int64) * P
    w_sym = int((ihi - starts).max())
    w_full = int((ihi - ilo).max())

    mirror = False
    if w_sym <= 2048:
        # symmetric mode: forward-only window [128b, ihi), host mirrors
        mirror = True
        W = max(512, -(-w_sym // 512) * 512)
        off = np.minimum(starts, N - W)
    elif w_full < N:
        W = max(2048, -(-w_full // 2048) * 2048)
        if W >= N:
            W = N
            order = np.arange(N)
            ps = posf
            ilo = np.zeros(nblocks, dtype=np.int64)
        off = np.clip(ilo, 0, N - W).astype(np.int64)
    else:
        W = N
        order = np.arange(N)
        ps = posf
        off = np.zeros(nblocks, dtype=np.int64)

    qrows, krows = _build_rows(ps)           # (32, N) f64 over sorted order
    q16 = qrows.astype(BF16)
    k16 = krows.astype(BF16)

    CH = W // 512
    WQ = 512 * ((CH + 3) // 4)
    nslab = BLOCKS if W < N else 1
    in_maps = []
    for c in range(NCORES):
        qc = np.zeros((P, BLOCKS, P), dtype=BF16)
        kc = np.zeros((P, nslab, WQ), dtype=BF16)
        for b in range(BLOCKS):
            g = c * BLOCKS + b
            qb = q16[:, g * P : (g + 1) * P]          # [32, 128]
            qc[:, b, :] = np.tile(qb, (4, 1))         # replicate to 4 groups
            if nslab > 1:
                kc[:, b, :] = _chunk_layout(k16[:, off[g] : off[g] + W], WQ)
        if nslab == 1:
            kc[:, 0, :] = _chunk_layout(k16, WQ)
        in_maps.append({"q": qc, "k": kc})

    slab_of_block = list(range(BLOCKS)) if nslab > 1 else [0] * BLOCKS
    return order, off, W, nslab, slab_of_block, in_maps, mirror


LAST_RESULTS = None  # BassKernelResults of the most recent run (for profiling)


def kernel(pos):
    global LAST_RESULTS
    order, off, W, nslab, slab_of_block, in_maps, mirror = _prepare(pos)
    nc = _build_graph(W, nslab, slab_of_block)
    res = run_bass_kernel_spmd(nc, in_maps, list(range(NCORES)))
    LAST_RESULTS = res

    full = np.zeros((N, N), dtype=bool)
    for c in range(NCORES):
        o = res.results[c]["out"]                      # [BLOCKS, 128, W] int8
        for b in range(BLOCKS):
            g = c * BLOCKS + b
            rows = order[g * P : (g + 1) * P]
            cols = order[off[g] : off[g] + W]
            full[np.ix_(rows, cols)] = o[b] == 1
    if mirror:
        full |= full.T
    return full


# revision 11
# speedup vs baseline: 1.3901x; 1.0276x over previous
"""Radius-graph adjacency mask (radius_graph r=3, loop=True) on 8 TRN2 NeuronCores.

Strategy
--------
mask[i, j] = (||p_i - p_j||^2 <= R2)  for pos [8192, 3].

val(i, j) = (R2 + eps) - d2(i, j) is computed as a single small-K matmul:
    val = sum_r q_rows[r, i] * k_rows[r, j]
where the q/k rows hold 3-way bf16 splits of the augmented query/key vectors
(2x, sq terms), so the bf16 TensorE matmul (1 cycle/row) reproduces the fp32
value to ~24-bit accuracy.  PSUM holds val; mask = (val >= 0) via VectorE
is_ge / ScalarE Sign (both engines split the PSUM-read load), written as int8
and DMA'd out.

Sharding: rows data-parallel across 8 cores (1024 query rows each).  Atoms are
z-sorted; in symmetric mode each 128-query block computes only keys at sorted
index >= its own start inside the z-window (all forward |z_i - z_j| <= 3
neighbors) — a W~1024 slab — and the host mirrors the lower triangle.  Each
core holds ONE shared key window [128*8c, 128*8c + WC); block b reads columns
[128b, 128b + W) of it.  The host scatters the slabs into the full mask.
"""

from contextlib import ExitStack

import ml_dtypes
import numpy as np

import concourse.bass as bass
import concourse.mybir as mybir
import concourse.tile as tile
from concourse import bacc
from concourse.bass_utils import run_bass_kernel_spmd

N = 8192
R2 = 9.0
RADIUS = 3.0
EPS = 1e-5
NCORES = 8
P = 128
KP = 32                       # padded contraction rows (30 used)
BLOCKS = (N // NCORES) // P   # 8 query blocks of 128 rows per core
BF16 = ml_dtypes.bfloat16

# threshold engine per block slot: ScalarE Sign is cheaper than VectorE is_ge,
# so ACT takes 5 of 8 (incl. the last block, which gates the output tail)
ACT_BLOCKS = (0, 2, 4, 6, 7)


def _bf16_split3(x):
    """Split f64 array into 3 bf16 components summing to ~24-bit accuracy."""
    b0 = x.astype(BF16)
    r1 = x - b0.astype(np.float64)
    b1 = r1.astype(BF16)
    r2 = r1 - b1.astype(np.float64)
    b2 = r2.astype(BF16)
    return b0.astype(np.float64), b1.astype(np.float64), b2.astype(np.float64)


def _build_rows(ps):
    """Build the KP-row augmented query/key matrices (f64 holding bf16 values).

    val = sum_r q_rows[r, i] * k_rows[r, j] = (R2 + EPS) - d2(i, j)
    """
    n = ps.shape[0]
    A = 2.0 * ps.T                      # (3, n) query-side coefficient
    B = ps.T                            # (3, n) key-side
    S = (R2 + EPS) - (ps * ps).sum(1)   # query-side constant term
    T = -(ps * ps).sum(1)               # key-side constant term
    ones = np.ones(n)

    rows_q, rows_k = [], []
    for c in range(3):
        Asp = _bf16_split3(A[c])
        Bsp = _bf16_split3(B[c])
        # all split-product terms above ~2^-32 relative (drop (2,2) only)
        for u, v in [(0, 0), (0, 1), (1, 0), (1, 1), (0, 2), (2, 0), (1, 2), (2, 1)]:
            rows_q.append(Asp[u])
            rows_k.append(Bsp[v])
    for s in _bf16_split3(S):
        rows_q.append(s)
        rows_k.append(ones)
    for t in _bf16_split3(T):
        rows_q.append(ones)
        rows_k.append(t)

    q = np.zeros((KP, n))
    k = np.zeros((KP, n))
    q[: len(rows_q)] = np.stack(rows_q)
    k[: len(rows_k)] = np.stack(rows_k)
    return q, k


def _build_graph_shared(W, WC):
    """Symmetric-mode SPMD graph: one shared key window per core.

    Inputs (per core):
      q [128, BLOCKS, 128] bf16 : partition 32g + r holds q_rows[r] for the
                                  block's queries, replicated over g=0..3.
      k [128, WC] bf16          : partition 32g + r holds k_rows[r] for the
                                  core window, replicated over g=0..3.
    Output:
      out [BLOCKS, 128, W] int8 : 1 where mask else 0/-1 (decode == 1).
    """
    assert W % 512 == 0 and W <= 2048
    NT = W // 512                      # matmul N-tiles per block
    GP = 2                             # blocks per output DMA

    nc = bacc.Bacc("TRN2", target_bir_lowering=False)
    q_ext = nc.declare_dram_parameter("q", [P, BLOCKS, P], mybir.dt.bfloat16, isOutput=False)
    k_ext = nc.declare_dram_parameter("k", [P, WC], mybir.dt.bfloat16, isOutput=False)
    out_ext = nc.declare_dram_parameter("out", [BLOCKS, P, W], mybir.dt.int8, isOutput=True)

    with tile.TileContext(nc) as tc, ExitStack() as ctx:
        kpool = ctx.enter_context(tc.tile_pool(name="keys", bufs=1))
        qpool = ctx.enter_context(tc.tile_pool(name="queries", bufs=1))
        psum = ctx.enter_context(tc.tile_pool(name="psum", bufs=8 // NT, space="PSUM"))
        mpool = ctx.enter_context(tc.tile_pool(name="mask", bufs=2))

        # q on the sync queue; k split in two on the scalar queue so block 0's
        # matmuls only wait for the first half
        q_sb = qpool.tile([P, BLOCKS, P], mybir.dt.bfloat16)
        nc.sync.dma_start(q_sb[:], q_ext[:])
        k_sb = kpool.tile([P, WC], mybir.dt.bfloat16)
        nc.scalar.dma_start(k_sb[:, :W], k_ext[:, :W])
        nc.scalar.dma_start(k_sb[:, W:], k_ext[:, W:])

        mt = None
        for b in range(BLOCKS):
            pt = psum.tile([P, W], mybir.dt.float32)
            for t in range(NT):
                g = (NT * b + t) % 4           # PE row group (spread load)
                col = P * b + 512 * t          # column offset in core window
                nc.tensor.matmul(
                    pt[:, 512 * t : 512 * (t + 1)],
                    lhsT=q_sb[32 * g : 32 * (g + 1), b, :],
                    rhs=k_sb[32 * g : 32 * (g + 1), col : col + 512],
                    start=True,
                    stop=True,
                    tile_position=(32 * g, 0),
                )
            gslot = b % GP
            if gslot == 0:
                mt = mpool.tile([P, GP, W], mybir.dt.int8)
            if b in ACT_BLOCKS:
                nc.scalar.activation(mt[:, gslot], pt[:], mybir.ActivationFunctionType.Sign)
            else:
                nc.vector.tensor_scalar(mt[:, gslot], pt[:], 0.0, None, mybir.AluOpType.is_ge)
            if gslot == GP - 1:
                nc.sync.dma_start(
                    out_ext[b - GP + 1 : b + 1, :, :].rearrange("b p w -> p b w"),
                    mt[:],
                )
    nc.compile()
    return nc


def _build_graph_slabs(W, nslab, slab_of_block):
    """Fallback SPMD graph: per-block key slabs (non-symmetric windows / dense).

    Column chunk c (of W//512) lives at PE row group c%4, free offset (c//4)*512.
    """
    CH = W // 512
    WQ = 512 * ((CH + 3) // 4)
    PSUM_FD = W if W <= 2048 else 2048
    tiles_per_block = W // PSUM_FD
    GP = 2

    nc = bacc.Bacc("TRN2", target_bir_lowering=False)
    q_ext = nc.declare_dram_parameter("q", [P, BLOCKS, P], mybir.dt.bfloat16, isOutput=False)
    k_ext = nc.declare_dram_parameter("k", [P, nslab, WQ], mybir.dt.bfloat16, isOutput=False)
    out_ext = nc.declare_dram_parameter("out", [BLOCKS, P, W], mybir.dt.int8, isOutput=True)

    with tile.TileContext(nc) as tc, ExitStack() as ctx:
        kpool = ctx.enter_context(tc.tile_pool(name="keys", bufs=1))
        qpool = ctx.enter_context(tc.tile_pool(name="queries", bufs=1))
        psum = ctx.enter_context(
            tc.tile_pool(name="psum", bufs=8 // (PSUM_FD // 512), space="PSUM")
        )
        mpool = ctx.enter_context(tc.tile_pool(name="mask", bufs=2))

        q_sb = qpool.tile([P, BLOCKS, P], mybir.dt.bfloat16)
        nc.sync.dma_start(q_sb[:], q_ext[:])
        k_sb = kpool.tile([P, nslab, WQ], mybir.dt.bfloat16)
        nc.scalar.dma_start(k_sb[:], k_ext[:])

        tidx = 0
        mt = None
        for b in range(BLOCKS):
            s = slab_of_block[b]
            for h in range(tiles_per_block):
                pt = psum.tile([P, PSUM_FD], mybir.dt.float32)
                for t2 in range(PSUM_FD // 512):
                    c = h * (PSUM_FD // 512) + t2
                    g = c % 4
                    qcol = (c // 4) * 512
                    nc.tensor.matmul(
                        pt[:, t2 * 512 : (t2 + 1) * 512],
                        lhsT=q_sb[32 * g : 32 * (g + 1), b, :],
                        rhs=k_sb[32 * g : 32 * (g + 1), s, qcol : qcol + 512],
                        start=True,
                        stop=True,
                        tile_position=(32 * g, 0),
                    )
                gslot = tidx % GP
                if gslot == 0:
                    mt = mpool.tile([P, GP, PSUM_FD], mybir.dt.int8)
                if tidx % 2 == 0:
                    nc.scalar.activation(mt[:, gslot], pt[:], mybir.ActivationFunctionType.Sign)
                else:
                    nc.vector.tensor_scalar(mt[:, gslot], pt[:], 0.0, None, mybir.AluOpType.is_ge)
                if gslot == GP - 1:
                    if tiles_per_block == 1:
                        nc.sync.dma_start(
                            out_ext[b - GP + 1 : b + 1, :, :].rearrange("b p w -> p b w"),
                            mt[:],
                        )
                    else:
                        nc.sync.dma_start(
                            out_ext[b, :, (h - GP + 1) * PSUM_FD : (h + 1) * PSUM_FD],
                            mt[:].rearrange("p b w -> p (b w)"),
                        )
                tidx += 1
    nc.compile()
    return nc


def _chunk_layout(k32, WQ):
    """[32, W] -> [128, WQ]: chunk c (512 cols) at partitions 32*(c%4), offset (c//4)*512."""
    W = k32.shape[1]
    out = np.zeros((P, WQ), dtype=k32.dtype)
    for c in range(W // 512):
        g, m = c % 4, c // 4
        out[32 * g : 32 * (g + 1), m * 512 : (m + 1) * 512] = k32[:, c * 512 : (c + 1) * 512]
    return out


def _prepare(pos):
    """Host prep: sort, windows, split rows, per-core in_maps."""
    posf = np.asarray(pos, dtype=np.float64)
    nblocks = N // P

    # recenter: d2 is translation-invariant, but smaller |coords| shrink the
    # fp32 cancellation error in sq_i + sq_j - 2 x.y by ~4x
    posf = posf - (posf.min(0) + posf.max(0)) / 2.0

    order = np.argsort(posf[:, 2], kind="stable")
    ps = posf[order]
    z = ps[:, 2]
    zb = z.reshape(nblocks, P)
    ilo = np.searchsorted(z, zb.min(1) - RADIUS, side="left")
    ihi = np.searchsorted(z, zb.max(1) + RADIUS, side="right")

    starts = np.arange(nblocks, dtype=np.int64) * P
    w_sym = int((ihi - starts).max())

    if w_sym <= 2048:
        # symmetric mode: block window [128b, 128b + W), host mirrors
        W = max(512, -(-w_sym // 512) * 512)
        WC = P * (BLOCKS - 1) + W
        qrows, krows = _build_rows(ps)
        q16 = qrows.astype(BF16)
        # pad key tail with far-away dummies (mask always 0 there)
        k16 = np.zeros((KP, N + WC), dtype=BF16)
        k16[:, :N] = krows.astype(BF16)
        k16[KP - 3, N:] = -1e9          # T0 row: val = S_i - 1e9 < 0
        in_maps = []
        for c in range(NCORES):
            qc = np.zeros((P, BLOCKS, P), dtype=BF16)
            for b in range(BLOCKS):
                g = c * BLOCKS + b
                qc[:, b, :] = np.tile(q16[:, g * P : (g + 1) * P], (4, 1))
            coff = c * BLOCKS * P
            kc = np.tile(k16[:, coff : coff + WC], (4, 1))
            in_maps.append({"q": qc, "k": kc})
        return order, W, WC, in_maps, True, None

    # fallback: non-symmetric z-window slabs, or dense
    w_full = int((ihi - ilo).max())
    if w_full < N:
        W = max(2048, -(-w_full // 2048) * 2048)
    else:
        W = N
    if W >= N:
        W = N
        order = np.arange(N)
        ps = posf
        ilo = np.zeros(nblocks, dtype=np.int64)
    off = np.clip(ilo, 0, N - W).astype(np.int64)

    qrows, krows = _build_rows(ps)
    q16 = qrows.astype(BF16)
    k16 = krows.astype(BF16)
    WQ = 512 * ((W // 512 + 3) // 4)
    nslab = BLOCKS if W < N else 1
    in_maps = []
    for c in range(NCORES):
        qc = np.zeros((P, BLOCKS, P), dtype=BF16)
        kc = np.zeros((P, nslab, WQ), dtype=BF16)
        for b in range(BLOCKS):
            g = c * BLOCKS + b
            qc[:, b, :] = np.tile(q16[:, g * P : (g + 1) * P], (4, 1))
            if nslab > 1:
                kc[:, b, :] = _chunk_layout(k16[:, off[g] : off[g] + W], WQ)
        if nslab == 1:
            kc[:, 0, :] = _chunk_layout(k16, WQ)
        in_maps.append({"q": qc, "k": kc})
    slab_of_block = list(range(BLOCKS)) if nslab > 1 else [0] * BLOCKS
    return order, W, (nslab, slab_of_block, off), in_maps, False, None


LAST_RESULTS = None  # BassKernelResults of the most recent run (for profiling)


def kernel(pos):
    global LAST_RESULTS
    order, W, extra, in_maps, mirror, _ = _prepare(pos)
    if mirror:
        WC = extra
        nc = _build_graph_shared(W, WC)
    else:
        nslab, slab_of_block, off = extra
        nc = _build_graph_slabs(W, nslab, slab_of_block)
    res = run_bass_kernel_spmd(nc, in_maps, list(range(NCORES)))
    LAST_RESULTS = res

    full = np.zeros((N, N), dtype=bool)
    for c in range(NCORES):
        o = res.results[c]["out"]                      # [BLOCKS, 128, W] int8
        for b in range(BLOCKS):
            g = c * BLOCKS + b
            rows = order[g * P : (g + 1) * P]
            start = g * P if mirror else off[g]
            valid = min(W, N - start)
            cols = order[start : start + valid]
            full[np.ix_(rows, cols)] = o[b][:, :valid] == 1
    if mirror:
        full |= full.T
    return full


# revision 12
# speedup vs baseline: 1.4011x; 1.0080x over previous
"""Radius-graph adjacency mask (radius_graph r=3, loop=True) on 8 TRN2 NeuronCores.

Strategy
--------
mask[i, j] = (||p_i - p_j||^2 <= R2)  for pos [8192, 3].

val(i, j) = (R2 + eps) - d2(i, j) is computed as a single small-K matmul:
    val = sum_r q_rows[r, i] * k_rows[r, j]
where the q/k rows hold 3-way bf16 splits of the augmented query/key vectors
(2x, sq terms), so the bf16 TensorE matmul (1 cycle/row) reproduces the fp32
value to ~24-bit accuracy.  PSUM holds val; mask = (val >= 0) via VectorE
is_ge / ScalarE Sign (both engines split the PSUM-read load), written as int8
and DMA'd out.

Sharding: rows data-parallel across 8 cores (1024 query rows each).  Atoms are
z-sorted; in symmetric mode each 128-query block computes only keys at sorted
index >= its own start inside the z-window (all forward |z_i - z_j| <= 3
neighbors) — a W~1024 slab — and the host mirrors the lower triangle.  Each
core holds ONE shared key window [128*8c, 128*8c + WC); block b reads columns
[128b, 128b + W) of it.  The host scatters the slabs into the full mask.
"""

from contextlib import ExitStack

import ml_dtypes
import numpy as np

import concourse.bass as bass
import concourse.mybir as mybir
import concourse.tile as tile
from concourse import bacc
from concourse.bass_utils import run_bass_kernel_spmd

N = 8192
R2 = 9.0
RADIUS = 3.0
EPS = 1e-5
NCORES = 8
P = 128
KP = 32                       # padded contraction rows (30 used)
BLOCKS = (N // NCORES) // P   # 8 query blocks of 128 rows per core
BF16 = ml_dtypes.bfloat16

# threshold engine per block slot: ScalarE Sign is cheaper than VectorE is_ge,
# so ACT takes 5 of 8 (incl. the last block, which gates the output tail)
ACT_BLOCKS = (0, 2, 4, 6, 7)


def _bf16_split3(x):
    """Split f64 array into 3 bf16 components summing to ~24-bit accuracy."""
    b0 = x.astype(BF16)
    r1 = x - b0.astype(np.float64)
    b1 = r1.astype(BF16)
    r2 = r1 - b1.astype(np.float64)
    b2 = r2.astype(BF16)
    return b0.astype(np.float64), b1.astype(np.float64), b2.astype(np.float64)


def _build_rows(ps):
    """Build the KP-row augmented query/key matrices (f64 holding bf16 values).

    val = sum_r q_rows[r, i] * k_rows[r, j] = (R2 + EPS) - d2(i, j)
    """
    n = ps.shape[0]
    A = 2.0 * ps.T                      # (3, n) query-side coefficient
    B = ps.T                            # (3, n) key-side
    S = (R2 + EPS) - (ps * ps).sum(1)   # query-side constant term
    T = -(ps * ps).sum(1)               # key-side constant term
    ones = np.ones(n)

    rows_q, rows_k = [], []
    for c in range(3):
        Asp = _bf16_split3(A[c])
        Bsp = _bf16_split3(B[c])
        # all split-product terms above ~2^-32 relative (drop (2,2) only)
        for u, v in [(0, 0), (0, 1), (1, 0), (1, 1), (0, 2), (2, 0), (1, 2), (2, 1)]:
            rows_q.append(Asp[u])
            rows_k.append(Bsp[v])
    for s in _bf16_split3(S):
        rows_q.append(s)
        rows_k.append(ones)
    for t in _bf16_split3(T):
        rows_q.append(ones)
        rows_k.append(t)

    q = np.zeros((KP, n))
    k = np.zeros((KP, n))
    q[: len(rows_q)] = np.stack(rows_q)
    k[: len(rows_k)] = np.stack(rows_k)
    return q, k


def _build_graph_shared(W, WC):
    """Symmetric-mode SPMD graph: one shared key window per core.

    Inputs (per core):
      q [128, BLOCKS, 128] bf16 : partition 32g + r holds q_rows[r] for the
                                  block's queries, replicated over g=0..3.
      k [128, WC] bf16          : partition 32g + r holds k_rows[r] for the
                                  core window, replicated over g=0..3.
    Output:
      out [BLOCKS, 128, W] int8 : 1 where mask else 0/-1 (decode == 1).
    """
    assert W % 512 == 0 and W <= 2048
    NT = W // 512                      # matmul N-tiles per block
    GP = 2                             # blocks per output DMA

    nc = bacc.Bacc("TRN2", target_bir_lowering=False)
    q_ext = nc.declare_dram_parameter("q", [P, BLOCKS, P], mybir.dt.bfloat16, isOutput=False)
    k_ext = nc.declare_dram_parameter("k", [P, WC], mybir.dt.bfloat16, isOutput=False)
    out_ext = nc.declare_dram_parameter("out", [BLOCKS, P, W], mybir.dt.int8, isOutput=True)

    with tile.TileContext(nc) as tc, ExitStack() as ctx:
        kpool = ctx.enter_context(tc.tile_pool(name="keys", bufs=1))
        qpool = ctx.enter_context(tc.tile_pool(name="queries", bufs=1))
        psum = ctx.enter_context(tc.tile_pool(name="psum", bufs=8 // NT, space="PSUM"))
        mpool = ctx.enter_context(tc.tile_pool(name="mask", bufs=BLOCKS // 2))

        # q on the sync queue (block 0's slice first); k split in two on the
        # scalar queue so block 0's matmuls only wait for the first half
        q_sb = qpool.tile([P, BLOCKS, P], mybir.dt.bfloat16)
        nc.sync.dma_start(q_sb[:, :1], q_ext[:, :1])
        nc.sync.dma_start(q_sb[:, 1:], q_ext[:, 1:])
        k_sb = kpool.tile([P, WC], mybir.dt.bfloat16)
        nc.scalar.dma_start(k_sb[:, :W], k_ext[:, :W])
        nc.scalar.dma_start(k_sb[:, W:], k_ext[:, W:])

        mt = None
        for b in range(BLOCKS):
            pt = psum.tile([P, W], mybir.dt.float32)
            for t in range(NT):
                g = (NT * b + t) % 4           # PE row group (spread load)
                col = P * b + 512 * t          # column offset in core window
                nc.tensor.matmul(
                    pt[:, 512 * t : 512 * (t + 1)],
                    lhsT=q_sb[32 * g : 32 * (g + 1), b, :],
                    rhs=k_sb[32 * g : 32 * (g + 1), col : col + 512],
                    start=True,
                    stop=True,
                    tile_position=(32 * g, 0),
                )
            gslot = b % GP
            if gslot == 0:
                mt = mpool.tile([P, GP, W], mybir.dt.int8)
            if b in ACT_BLOCKS:
                nc.scalar.activation(mt[:, gslot], pt[:], mybir.ActivationFunctionType.Sign)
            else:
                nc.vector.tensor_scalar(mt[:, gslot], pt[:], 0.0, None, mybir.AluOpType.is_ge)
            if gslot == GP - 1:
                nc.sync.dma_start(
                    out_ext[b - GP + 1 : b + 1, :, :].rearrange("b p w -> p b w"),
                    mt[:],
                )
    nc.compile()
    return nc


def _build_graph_slabs(W, nslab, slab_of_block):
    """Fallback SPMD graph: per-block key slabs (non-symmetric windows / dense).

    Column chunk c (of W//512) lives at PE row group c%4, free offset (c//4)*512.
    """
    CH = W // 512
    WQ = 512 * ((CH + 3) // 4)
    PSUM_FD = W if W <= 2048 else 2048
    tiles_per_block = W // PSUM_FD
    GP = 2

    nc = bacc.Bacc("TRN2", target_bir_lowering=False)
    q_ext = nc.declare_dram_parameter("q", [P, BLOCKS, P], mybir.dt.bfloat16, isOutput=False)
    k_ext = nc.declare_dram_parameter("k", [P, nslab, WQ], mybir.dt.bfloat16, isOutput=False)
    out_ext = nc.declare_dram_parameter("out", [BLOCKS, P, W], mybir.dt.int8, isOutput=True)

    with tile.TileContext(nc) as tc, ExitStack() as ctx:
        kpool = ctx.enter_context(tc.tile_pool(name="keys", bufs=1))
        qpool = ctx.enter_context(tc.tile_pool(name="queries", bufs=1))
        psum = ctx.enter_context(
            tc.tile_pool(name="psum", bufs=8 // (PSUM_FD // 512), space="PSUM")
        )
        mpool = ctx.enter_context(tc.tile_pool(name="mask", bufs=2))

        q_sb = qpool.tile([P, BLOCKS, P], mybir.dt.bfloat16)
        nc.sync.dma_start(q_sb[:], q_ext[:])
        k_sb = kpool.tile([P, nslab, WQ], mybir.dt.bfloat16)
        nc.scalar.dma_start(k_sb[:], k_ext[:])

        tidx = 0
        mt = None
        for b in range(BLOCKS):
            s = slab_of_block[b]
            for h in range(tiles_per_block):
                pt = psum.tile([P, PSUM_FD], mybir.dt.float32)
                for t2 in range(PSUM_FD // 512):
                    c = h * (PSUM_FD // 512) + t2
                    g = c % 4
                    qcol = (c // 4) * 512
                    nc.tensor.matmul(
                        pt[:, t2 * 512 : (t2 + 1) * 512],
                        lhsT=q_sb[32 * g : 32 * (g + 1), b, :],
                        rhs=k_sb[32 * g : 32 * (g + 1), s, qcol : qcol + 512],
                        start=True,
                        stop=True,
                        tile_position=(32 * g, 0),
                    )
                gslot = tidx % GP
                if gslot == 0:
                    mt = mpool.tile([P, GP, PSUM_FD], mybir.dt.int8)
                if tidx % 2 == 0:
                    nc.scalar.activation(mt[:, gslot], pt[:], mybir.ActivationFunctionType.Sign)
                else:
                    nc.vector.tensor_scalar(mt[:, gslot], pt[:], 0.0, None, mybir.AluOpType.is_ge)
                if gslot == GP - 1:
                    if tiles_per_block == 1:
                        nc.sync.dma_start(
                            out_ext[b - GP + 1 : b + 1, :, :].rearrange("b p w -> p b w"),
                            mt[:],
                        )
                    else:
                        nc.sync.dma_start(
                            out_ext[b, :, (h - GP + 1) * PSUM_FD : (h + 1) * PSUM_FD],
                            mt[:].rearrange("p b w -> p (b w)"),
                        )
                tidx += 1
    nc.compile()
    return nc


def _chunk_layout(k32, WQ):
    """[32, W] -> [128, WQ]: chunk c (512 cols) at partitions 32*(c%4), offset (c//4)*512."""
    W = k32.shape[1]
    out = np.zeros((P, WQ), dtype=k32.dtype)
    for c in range(W // 512):
        g, m = c % 4, c // 4
        out[32 * g : 32 * (g + 1), m * 512 : (m + 1) * 512] = k32[:, c * 512 : (c + 1) * 512]
    return out


def _prepare(pos):
    """Host prep: sort, windows, split rows, per-core in_maps."""
    posf = np.asarray(pos, dtype=np.float64)
    nblocks = N // P

    # recenter: d2 is translation-invariant, but smaller |coords| shrink the
    # fp32 cancellation error in sq_i + sq_j - 2 x.y by ~4x
    posf = posf - (posf.min(0) + posf.max(0)) / 2.0

    order = np.argsort(posf[:, 2], kind="stable")
    ps = posf[order]
    z = ps[:, 2]
    zb = z.reshape(nblocks, P)
    ilo = np.searchsorted(z, zb.min(1) - RADIUS, side="left")
    ihi = np.searchsorted(z, zb.max(1) + RADIUS, side="right")

    starts = np.arange(nblocks, dtype=np.int64) * P
    w_sym = int((ihi - starts).max())

    if w_sym <= 2048:
        # symmetric mode: block window [128b, 128b + W), host mirrors
        W = max(512, -(-w_sym // 512) * 512)
        WC = P * (BLOCKS - 1) + W
        qrows, krows = _build_rows(ps)
        q16 = qrows.astype(BF16)
        # pad key tail with far-away dummies (mask always 0 there)
        k16 = np.zeros((KP, N + WC), dtype=BF16)
        k16[:, :N] = krows.astype(BF16)
        k16[KP - 3, N:] = -1e9          # T0 row: val = S_i - 1e9 < 0
        in_maps = []
        for c in range(NCORES):
            qc = np.zeros((P, BLOCKS, P), dtype=BF16)
            for b in range(BLOCKS):
                g = c * BLOCKS + b
                qc[:, b, :] = np.tile(q16[:, g * P : (g + 1) * P], (4, 1))
            coff = c * BLOCKS * P
            kc = np.tile(k16[:, coff : coff + WC], (4, 1))
            in_maps.append({"q": qc, "k": kc})
        return order, W, WC, in_maps, True, None

    # fallback: non-symmetric z-window slabs, or dense
    w_full = int((ihi - ilo).max())
    if w_full < N:
        W = max(2048, -(-w_full // 2048) * 2048)
    else:
        W = N
    if W >= N:
        W = N
        order = np.arange(N)
        ps = posf
        ilo = np.zeros(nblocks, dtype=np.int64)
    off = np.clip(ilo, 0, N - W).astype(np.int64)

    qrows, krows = _build_rows(ps)
    q16 = qrows.astype(BF16)
    k16 = krows.astype(BF16)
    WQ = 512 * ((W // 512 + 3) // 4)
    nslab = BLOCKS if W < N else 1
    in_maps = []
    for c in range(NCORES):
        qc = np.zeros((P, BLOCKS, P), dtype=BF16)
        kc = np.zeros((P, nslab, WQ), dtype=BF16)
        for b in range(BLOCKS):
            g = c * BLOCKS + b
            qc[:, b, :] = np.tile(q16[:, g * P : (g + 1) * P], (4, 1))
            if nslab > 1:
                kc[:, b, :] = _chunk_layout(k16[:, off[g] : off[g] + W], WQ)
        if nslab == 1:
            kc[:, 0, :] = _chunk_layout(k16, WQ)
        in_maps.append({"q": qc, "k": kc})
    slab_of_block = list(range(BLOCKS)) if nslab > 1 else [0] * BLOCKS
    return order, W, (nslab, slab_of_block, off), in_maps, False, None


LAST_RESULTS = None  # BassKernelResults of the most recent run (for profiling)


def kernel(pos):
    global LAST_RESULTS
    order, W, extra, in_maps, mirror, _ = _prepare(pos)
    if mirror:
        WC = extra
        nc = _build_graph_shared(W, WC)
    else:
        nslab, slab_of_block, off = extra
        nc = _build_graph_slabs(W, nslab, slab_of_block)
    res = run_bass_kernel_spmd(nc, in_maps, list(range(NCORES)))
    LAST_RESULTS = res

    full = np.zeros((N, N), dtype=bool)
    for c in range(NCORES):
        o = res.results[c]["out"]                      # [BLOCKS, 128, W] int8
        for b in range(BLOCKS):
            g = c * BLOCKS + b
            rows = order[g * P : (g + 1) * P]
            start = g * P if mirror else off[g]
            valid = min(W, N - start)
            cols = order[start : start + valid]
            full[np.ix_(rows, cols)] = o[b][:, :valid] == 1
    if mirror:
        full |= full.T
    return full


# revision 15
# speedup vs baseline: 1.5087x; 1.0768x over previous
"""Radius-graph adjacency mask (radius_graph r=3, loop=True) on 8 TRN2 NeuronCores.

Strategy
--------
mask[i, j] = (||p_i - p_j||^2 <= R2)  for pos [8192, 3].

val(i, j) = (R2 + eps) - d2(i, j) is computed as a single small-K matmul:
    val = sum_r q_rows[r, i] * k_rows[r, j]
where the q/k rows hold 3-way bf16 splits of the augmented query/key vectors
(2x, sq terms), so the bf16 TensorE matmul (1 cycle/row) reproduces the fp32
value to ~24-bit accuracy.  PSUM holds val; mask = (val >= 0) via VectorE
is_ge / ScalarE Sign (both engines split the PSUM-read load), written as int8
and DMA'd out.

Sharding: rows data-parallel across 8 cores (1024 query rows each).  Atoms are
z-sorted; in symmetric mode each 128-query block computes only keys at sorted
index >= its own start inside the z-window (all forward |z_i - z_j| <= 3
neighbors) — a W~1024 slab — and the host mirrors the lower triangle.  Each
core holds ONE shared key window [128*8c, 128*8c + WC); block b reads columns
[128b, 128b + W) of it.  The host scatters the slabs into the full mask.
"""

from contextlib import ExitStack

import ml_dtypes
import numpy as np

import concourse.bass as bass
import concourse.mybir as mybir
import concourse.tile as tile
from concourse import bacc
from concourse.bass_utils import run_bass_kernel_spmd

N = 8192
R2 = 9.0
RADIUS = 3.0
EPS = 1e-5
NCORES = 8
P = 128
KP = 32                       # padded contraction rows (30 used)
BLOCKS = (N // NCORES) // P   # 8 query blocks of 128 rows per core
BF16 = ml_dtypes.bfloat16

# threshold engine per block slot: ScalarE Sign is cheaper than VectorE is_ge,
# so ACT takes 5 of 8 (incl. the last block, which gates the output tail)
ACT_BLOCKS = (0, 2, 4, 6, 7)


def _bf16_split3(x):
    """Split f64 array into 3 bf16 components summing to ~24-bit accuracy."""
    b0 = x.astype(BF16)
    r1 = x - b0.astype(np.float64)
    b1 = r1.astype(BF16)
    r2 = r1 - b1.astype(np.float64)
    b2 = r2.astype(BF16)
    return b0.astype(np.float64), b1.astype(np.float64), b2.astype(np.float64)


def _build_rows(ps):
    """Build the KP-row augmented query/key matrices (f64 holding bf16 values).

    val = sum_r q_rows[r, i] * k_rows[r, j] = (R2 + EPS) - d2(i, j)
    """
    n = ps.shape[0]
    A = 2.0 * ps.T                      # (3, n) query-side coefficient
    B = ps.T                            # (3, n) key-side
    S = (R2 + EPS) - (ps * ps).sum(1)   # query-side constant term
    T = -(ps * ps).sum(1)               # key-side constant term
    ones = np.ones(n)

    rows_q, rows_k = [], []
    for c in range(3):
        Asp = _bf16_split3(A[c])
        Bsp = _bf16_split3(B[c])
        # all split-product terms above ~2^-32 relative (drop (2,2) only)
        for u, v in [(0, 0), (0, 1), (1, 0), (1, 1), (0, 2), (2, 0), (1, 2), (2, 1)]:
            rows_q.append(Asp[u])
            rows_k.append(Bsp[v])
    for s in _bf16_split3(S):
        rows_q.append(s)
        rows_k.append(ones)
    for t in _bf16_split3(T):
        rows_q.append(ones)
        rows_k.append(t)

    q = np.zeros((KP, n))
    k = np.zeros((KP, n))
    q[: len(rows_q)] = np.stack(rows_q)
    k[: len(rows_k)] = np.stack(rows_k)
    return q, k


def _build_graph_shared_raw(W, WC):
    """Raw Block version of the symmetric shared-window graph.

    Manual engine streams + semaphores (no TileContext): saves the Tile
    entry/exit drain + barrier + sem-clear machinery (~4us of exec window).

    Engine roles: sync = q DMA; scalar = k DMA + 4x Sign; vector = 4x is_ge;
    tensor = 16x matmul; gpsimd = 4x paired output DMA.
    """
    assert W % 512 == 0 and W <= 2048
    NT = W // 512

    nc = bacc.Bacc("TRN2", target_bir_lowering=False)
    q_ext = nc.declare_dram_parameter("q", [P, BLOCKS, P], mybir.dt.bfloat16, isOutput=False)
    k_ext = nc.declare_dram_parameter("k", [P, WC], mybir.dt.bfloat16, isOutput=False)
    out_ext = nc.declare_dram_parameter("out", [BLOCKS, P, W], mybir.dt.int8, isOutput=True)

    with ExitStack() as ctx:
        qsem = ctx.enter_context(nc.semaphore("qsem"))
        ksem = ctx.enter_context(nc.semaphore("ksem"))
        ksem2 = ctx.enter_context(nc.semaphore("ksem2"))
        pe_sem = ctx.enter_context(nc.semaphore("pe_sem"))
        act_sem = ctx.enter_context(nc.semaphore("act_sem"))
        dve_sem = ctx.enter_context(nc.semaphore("dve_sem"))
        osem = ctx.enter_context(nc.semaphore("osem"))
        q_sb = ctx.enter_context(nc.sbuf_tensor("q_sb", [P, BLOCKS, P], mybir.dt.bfloat16))
        k_sb = ctx.enter_context(nc.sbuf_tensor("k_sb", [P, WC], mybir.dt.bfloat16))
        masks = [
            ctx.enter_context(nc.sbuf_tensor(f"m{i}", [P, 2, W], mybir.dt.int8))
            for i in range(BLOCKS // 2)
        ]
        psums = [
            ctx.enter_context(nc.psum_tensor(f"ps{i}", [P, W], mybir.dt.float32))
            for i in range(4)
        ]

        with nc.Block() as block:

            @block.sync
            def _(sync):
                sync.dma_start(out=q_sb[:], in_=q_ext[:]).then_inc(qsem, 16)

            @block.scalar
            def _(scalar):
                scalar.dma_start(out=k_sb[:, :W], in_=k_ext[:, :W]).then_inc(ksem, 16)
                scalar.dma_start(out=k_sb[:, W:], in_=k_ext[:, W:]).then_inc(ksem2, 16)
                for i, b in enumerate(range(0, BLOCKS, 2)):
                    scalar.wait_ge(pe_sem, b + 1)
                    scalar.activation(
                        masks[b // 2][:, 0], psums[b % 4][:],
                        mybir.ActivationFunctionType.Sign,
                    ).then_inc(act_sem, 1)

            @block.vector
            def _(vector):
                for i, b in enumerate(range(1, BLOCKS, 2)):
                    vector.wait_ge(pe_sem, b + 1)
                    vector.tensor_scalar(
                        masks[b // 2][:, 1], psums[b % 4][:],
                        0.0, None, mybir.AluOpType.is_ge,
                    ).then_inc(dve_sem, 1)

            @block.tensor
            def _(tensor):
                tensor.wait_ge(qsem, 16)
                tensor.wait_ge(ksem, 16)
                for b in range(BLOCKS):
                    if b == 1:
                        tensor.wait_ge(ksem2, 16)
                    if b >= 4:  # psum slot reuse: wait for block b-4's threshold
                        prev = b - 4
                        if prev % 2 == 0:
                            tensor.wait_ge(act_sem, prev // 2 + 1)
                        else:
                            tensor.wait_ge(dve_sem, prev // 2 + 1)
                    for t in range(NT):
                        g = (NT * b + t) % 4
                        col = P * b + 512 * t
                        mm = tensor.matmul(
                            psums[b % 4][:, 512 * t : 512 * (t + 1)],
                            lhsT=q_sb[32 * g : 32 * (g + 1), b, :],
                            rhs=k_sb[32 * g : 32 * (g + 1), col : col + 512],
                            start=True,
                            stop=True,
                            tile_position=(32 * g, 0),
                        )
                        if t == NT - 1:
                            mm.then_inc(pe_sem, 1)

            @block.gpsimd
            def _(gpsimd):
                for i in range(BLOCKS // 2):
                    gpsimd.wait_ge(act_sem, i + 1)
                    gpsimd.wait_ge(dve_sem, i + 1)
                    gpsimd.dma_start(
                        out=out_ext[2 * i : 2 * i + 2, :, :].rearrange("b p w -> p b w"),
                        in_=masks[i][:],
                    ).then_inc(osem, 16)
                gpsimd.wait_ge(osem, 16 * (BLOCKS // 2))

    nc.compile()
    return nc


def _build_graph_shared(W, WC):
    """Symmetric-mode SPMD graph: one shared key window per core.

    Inputs (per core):
      q [128, BLOCKS, 128] bf16 : partition 32g + r holds q_rows[r] for the
                                  block's queries, replicated over g=0..3.
      k [128, WC] bf16          : partition 32g + r holds k_rows[r] for the
                                  core window, replicated over g=0..3.
    Output:
      out [BLOCKS, 128, W] int8 : 1 where mask else 0/-1 (decode == 1).
    """
    assert W % 512 == 0 and W <= 2048
    NT = W // 512                      # matmul N-tiles per block
    GP = 2                             # blocks per output DMA

    nc = bacc.Bacc("TRN2", target_bir_lowering=False)
    q_ext = nc.declare_dram_parameter("q", [P, BLOCKS, P], mybir.dt.bfloat16, isOutput=False)
    k_ext = nc.declare_dram_parameter("k", [P, WC], mybir.dt.bfloat16, isOutput=False)
    out_ext = nc.declare_dram_parameter("out", [BLOCKS, P, W], mybir.dt.int8, isOutput=True)

    with tile.TileContext(nc) as tc, ExitStack() as ctx:
        kpool = ctx.enter_context(tc.tile_pool(name="keys", bufs=1))
        qpool = ctx.enter_context(tc.tile_pool(name="queries", bufs=1))
        psum = ctx.enter_context(tc.tile_pool(name="psum", bufs=8 // NT, space="PSUM"))
        mpool = ctx.enter_context(tc.tile_pool(name="mask", bufs=BLOCKS // 2))

        # q on the sync queue (block 0's slice first); k split in two on the
        # scalar queue so block 0's matmuls only wait for the first half
        q_sb = qpool.tile([P, BLOCKS, P], mybir.dt.bfloat16)
        nc.sync.dma_start(q_sb[:, :1], q_ext[:, :1])
        nc.sync.dma_start(q_sb[:, 1:], q_ext[:, 1:])
        k_sb = kpool.tile([P, WC], mybir.dt.bfloat16)
        nc.scalar.dma_start(k_sb[:, :W], k_ext[:, :W])
        nc.scalar.dma_start(k_sb[:, W:], k_ext[:, W:])

        mt = None
        for b in range(BLOCKS):
            pt = psum.tile([P, W], mybir.dt.float32)
            for t in range(NT):
                g = (NT * b + t) % 4           # PE row group (spread load)
                col = P * b + 512 * t          # column offset in core window
                nc.tensor.matmul(
                    pt[:, 512 * t : 512 * (t + 1)],
                    lhsT=q_sb[32 * g : 32 * (g + 1), b, :],
                    rhs=k_sb[32 * g : 32 * (g + 1), col : col + 512],
                    start=True,
                    stop=True,
                    tile_position=(32 * g, 0),
                )
            gslot = b % GP
            if gslot == 0:
                mt = mpool.tile([P, GP, W], mybir.dt.int8)
            if b in ACT_BLOCKS:
                nc.scalar.activation(mt[:, gslot], pt[:], mybir.ActivationFunctionType.Sign)
            else:
                nc.vector.tensor_scalar(mt[:, gslot], pt[:], 0.0, None, mybir.AluOpType.is_ge)
            if gslot == GP - 1:
                nc.sync.dma_start(
                    out_ext[b - GP + 1 : b + 1, :, :].rearrange("b p w -> p b w"),
                    mt[:],
                )
    nc.compile()
    return nc


def _build_graph_slabs(W, nslab, slab_of_block):
    """Fallback SPMD graph: per-block key slabs (non-symmetric windows / dense).

    Column chunk c (of W//512) lives at PE row group c%4, free offset (c//4)*512.
    """
    CH = W // 512
    WQ = 512 * ((CH + 3) // 4)
    PSUM_FD = W if W <= 2048 else 2048
    tiles_per_block = W // PSUM_FD
    GP = 2

    nc = bacc.Bacc("TRN2", target_bir_lowering=False)
    q_ext = nc.declare_dram_parameter("q", [P, BLOCKS, P], mybir.dt.bfloat16, isOutput=False)
    k_ext = nc.declare_dram_parameter("k", [P, nslab, WQ], mybir.dt.bfloat16, isOutput=False)
    out_ext = nc.declare_dram_parameter("out", [BLOCKS, P, W], mybir.dt.int8, isOutput=True)

    with tile.TileContext(nc) as tc, ExitStack() as ctx:
        kpool = ctx.enter_context(tc.tile_pool(name="keys", bufs=1))
        qpool = ctx.enter_context(tc.tile_pool(name="queries", bufs=1))
        psum = ctx.enter_context(
            tc.tile_pool(name="psum", bufs=8 // (PSUM_FD // 512), space="PSUM")
        )
        mpool = ctx.enter_context(tc.tile_pool(name="mask", bufs=2))

        q_sb = qpool.tile([P, BLOCKS, P], mybir.dt.bfloat16)
        nc.sync.dma_start(q_sb[:], q_ext[:])
        k_sb = kpool.tile([P, nslab, WQ], mybir.dt.bfloat16)
        nc.scalar.dma_start(k_sb[:], k_ext[:])

        tidx = 0
        mt = None
        for b in range(BLOCKS):
            s = slab_of_block[b]
            for h in range(tiles_per_block):
                pt = psum.tile([P, PSUM_FD], mybir.dt.float32)
                for t2 in range(PSUM_FD // 512):
                    c = h * (PSUM_FD // 512) + t2
                    g = c % 4
                    qcol = (c // 4) * 512
                    nc.tensor.matmul(
                        pt[:, t2 * 512 : (t2 + 1) * 512],
                        lhsT=q_sb[32 * g : 32 * (g + 1), b, :],
                        rhs=k_sb[32 * g : 32 * (g + 1), s, qcol : qcol + 512],
                        start=True,
                        stop=True,
                        tile_position=(32 * g, 0),
                    )
                gslot = tidx % GP
                if gslot == 0:
                    mt = mpool.tile([P, GP, PSUM_FD], mybir.dt.int8)
                if tidx % 2 == 0:
                    nc.scalar.activation(mt[:, gslot], pt[:], mybir.ActivationFunctionType.Sign)
                else:
                    nc.vector.tensor_scalar(mt[:, gslot], pt[:], 0.0, None, mybir.AluOpType.is_ge)
                if gslot == GP - 1:
                    if tiles_per_block == 1:
                        nc.sync.dma_start(
                            out_ext[b - GP + 1 : b + 1, :, :].rearrange("b p w -> p b w"),
                            mt[:],
                        )
                    else:
                        nc.sync.dma_start(
                            out_ext[b, :, (h - GP + 1) * PSUM_FD : (h + 1) * PSUM_FD],
                            mt[:].rearrange("p b w -> p (b w)"),
                        )
                tidx += 1
    nc.compile()
    return nc


def _chunk_layout(k32, WQ):
    """[32, W] -> [128, WQ]: chunk c (512 cols) at partitions 32*(c%4), offset (c//4)*512."""
    W = k32.shape[1]
    out = np.zeros((P, WQ), dtype=k32.dtype)
    for c in range(W // 512):
        g, m = c % 4, c // 4
        out[32 * g : 32 * (g + 1), m * 512 : (m + 1) * 512] = k32[:, c * 512 : (c + 1) * 512]
    return out


def _prepare(pos):
    """Host prep: sort, windows, split rows, per-core in_maps."""
    posf = np.asarray(pos, dtype=np.float64)
    nblocks = N // P

    # recenter: d2 is translation-invariant, but smaller |coords| shrink the
    # fp32 cancellation error in sq_i + sq_j - 2 x.y by ~4x
    posf = posf - (posf.min(0) + posf.max(0)) / 2.0

    order = np.argsort(posf[:, 2], kind="stable")
    ps = posf[order]
    z = ps[:, 2]
    zb = z.reshape(nblocks, P)
    ilo = np.searchsorted(z, zb.min(1) - RADIUS, side="left")
    ihi = np.searchsorted(z, zb.max(1) + RADIUS, side="right")

    starts = np.arange(nblocks, dtype=np.int64) * P
    w_sym = int((ihi - starts).max())

    if w_sym <= 2048:
        # symmetric mode: block window [128b, 128b + W), host mirrors
        W = max(512, -(-w_sym // 512) * 512)
        WC = P * (BLOCKS - 1) + W
        qrows, krows = _build_rows(ps)
        q16 = qrows.astype(BF16)
        # pad key tail with far-away dummies (mask always 0 there)
        k16 = np.zeros((KP, N + WC), dtype=BF16)
        k16[:, :N] = krows.astype(BF16)
        k16[KP - 3, N:] = -1e9          # T0 row: val = S_i - 1e9 < 0
        in_maps = []
        for c in range(NCORES):
            qc = np.zeros((P, BLOCKS, P), dtype=BF16)
            for b in range(BLOCKS):
                g = c * BLOCKS + b
                qc[:, b, :] = np.tile(q16[:, g * P : (g + 1) * P], (4, 1))
            coff = c * BLOCKS * P
            kc = np.tile(k16[:, coff : coff + WC], (4, 1))
            in_maps.append({"q": qc, "k": kc})
        return order, W, WC, in_maps, True, None

    # fallback: non-symmetric z-window slabs, or dense
    w_full = int((ihi - ilo).max())
    if w_full < N:
        W = max(2048, -(-w_full // 2048) * 2048)
    else:
        W = N
    if W >= N:
        W = N
        order = np.arange(N)
        ps = posf
        ilo = np.zeros(nblocks, dtype=np.int64)
    off = np.clip(ilo, 0, N - W).astype(np.int64)

    qrows, krows = _build_rows(ps)
    q16 = qrows.astype(BF16)
    k16 = krows.astype(BF16)
    WQ = 512 * ((W // 512 + 3) // 4)
    nslab = BLOCKS if W < N else 1
    in_maps = []
    for c in range(NCORES):
        qc = np.zeros((P, BLOCKS, P), dtype=BF16)
        kc = np.zeros((P, nslab, WQ), dtype=BF16)
        for b in range(BLOCKS):
            g = c * BLOCKS + b
            qc[:, b, :] = np.tile(q16[:, g * P : (g + 1) * P], (4, 1))
            if nslab > 1:
                kc[:, b, :] = _chunk_layout(k16[:, off[g] : off[g] + W], WQ)
        if nslab == 1:
            kc[:, 0, :] = _chunk_layout(k16, WQ)
        in_maps.append({"q": qc, "k": kc})
    slab_of_block = list(range(BLOCKS)) if nslab > 1 else [0] * BLOCKS
    return order, W, (nslab, slab_of_block, off), in_maps, False, None


LAST_RESULTS = None  # BassKernelResults of the most recent run (for profiling)


def kernel(pos):
    global LAST_RESULTS
    order, W, extra, in_maps, mirror, _ = _prepare(pos)
    if mirror:
        WC = extra
        nc = _build_graph_shared_raw(W, WC)
    else:
        nslab, slab_of_block, off = extra
        nc = _build_graph_slabs(W, nslab, slab_of_block)
    res = run_bass_kernel_spmd(nc, in_maps, list(range(NCORES)))
    LAST_RESULTS = res

    full = np.zeros((N, N), dtype=bool)
    for c in range(NCORES):
        o = res.results[c]["out"]                      # [BLOCKS, 128, W] int8
        for b in range(BLOCKS):
            g = c * BLOCKS + b
            rows = order[g * P : (g + 1) * P]
            start = g * P if mirror else off[g]
            valid = min(W, N - start)
            cols = order[start : start + valid]
            full[np.ix_(rows, cols)] = o[b][:, :valid] == 1
    if mirror:
        full |= full.T
    return full
